# revision 56
# baseline (speedup 1.0000x reference)
"""AttentionBlock (GroupNorm -> 1x1 qkv -> full self-attention -> out-proj -> residual)
on Trainium2, data-parallel over batch across 8 NeuronCores.

Full input shapes (hardcoded):
  x        (32, 256, 32, 32) fp32
  gn_weight(256,) gn_bias (256,)
  w_qkv    (768, 256)  b_qkv (768,)
  w_out    (256, 256)  b_out (256,)

Per-core work: 4 batch elements. All large matmuls run in fp8e4 with
perf_mode=DoubleRow (both 128-deep k-tiles of the C=256 contraction in one
pass at 0.5 cycles/row). Weights are host-scaled by 16 so fp8 operands sit
near 1.0; the 1/sqrt(c) softmax scale and the 16*16 factors fold into the exp
scale (1/4096) and output scale (1/256). exp bias -2 keeps P in fp8e4 range
(TRN e4m3 saturates at +-240); softmax is invariant to it.

GroupNorm rsqrt = exp(-0.5*ln(var+eps)); the activation-table map is patched
at build time so Exp and Ln both resolve to natural_log_exp_and_others and
ACT never reloads its function table.

The schedule is engine-balanced and software-pipelined at batch scope:
  ACT : 16 exp evacs + out-proj evacs of the PREVIOUS batch (deferred y)
  DVE : GN stats, qk/v PSUM evacs of the NEXT batch, softmax normalize
  Pool: GN scalar chain + xn apply (next batch), residual adds (prev batch)
  PE  : everything matmul, ~25% busy at fp8 DoubleRow rates
"""

import functools
import numpy as np

NCORES = 8
B, C, H, W = 32, 256, 32, 32
HW = H * W
BPC = B // NCORES        # batches per core
G = 8                    # groups
GSZ = C // G             # 32 channels / group
EPS = 1e-5
CT = C // 128            # channel tiles = 2
TT = HW // 128           # position tiles = 8
NT = HW // 512           # free-dim (512) tiles = 2

SC = 16.0                # host weight scale
EXP_SCALE = 1.0 / (SC * 16.0)        # 1/256: undo the 16 on M, apply 1/sqrt(c)
EXP_BIAS = -2.0
OUT_SCALE = 1.0 / (SC * SC)          # 1/256

# packed fp8 weight tensor layout (elements per partition)
# M = 16*Wk^T Wq replaces q,k entirely: S^T = xn^T M xn; the per-t bias
# 16*(Wk^T bq).xn folds into the exp bias AP and all per-s/const terms
# cancel in the softmax ratio.
_W8_M = 0            # [2, 256]  (M^T packed like Wout^T)
_W8_WV = 512         # [2, 256]
_W8_WO = 1024        # [2, 256]
_W8_ONES = 1536      # [2, 128] all ones (DoubleRow denominator)
_W8_WVEC = 1792      # [2, 1]   16*Wk^T bq
_W8_TOT = 1794
# packed fp32 scalar tensor (GN indicators + biases), single DMA
_S_IND1 = 0          # 8
_S_IND2 = 8          # rows 0-3: 128
_S_BO = 136          # 2  (b_out + w_out@b_v per kt)
_S_GNWB = 138        # [2, 2]
_S_TOT = 142

_LOOP_N = 1


def _patch_act_tables():
    """Restrict Exp/Ln to the one table that holds both, so the act-table
    insertion pass cannot thrash between exp_and_others and natural_log.
    Table ids keep their act_info.json indices, so the hardware load is the
    real natural_log_exp_and_others set (which does contain Exp and Ln)."""
    import concourse.hw_specs as hs
    import concourse.bacc as bacc_mod
    from concourse import mybir

    if getattr(hs, "_attn_fp8_tbl_patch", False):
        return
    orig = hs.get_activation_tables

    @functools.cache
    def patched(arch):
        AF = mybir.ActivationFunctionType
        out = {}
        for name, fns in orig(arch).items():
            fns = set(fns)
            if name != "natural_log_exp_and_others":
                fns.discard(AF.Exp)
                fns.discard(AF.Ln)
            out[name] = fns
        return out

    hs.get_activation_tables = patched
    for mod in (bacc_mod,):
        if getattr(mod, "get_activation_tables", None) is orig:
            mod.get_activation_tables = patched
    hs._attn_fp8_tbl_patch = True


@functools.lru_cache(maxsize=None)
def _build(loop_n: int):
    import concourse.bacc as bacc
    import concourse.tile as tile
    from concourse import mybir

    _patch_act_tables()

    f32 = mybir.dt.float32
    f16 = mybir.dt.float16
    f8 = mybir.dt.float8e4
    AF = mybir.ActivationFunctionType
    OP = mybir.AluOpType
    DR = mybir.MatmulPerfMode.DoubleRow

    nc = bacc.Bacc("TRN2", target_bir_lowering=False, debug=False)

    x_d = nc.declare_dram_parameter("x", [BPC, 128, CT * HW], f32, isOutput=False)
    parw8_d = nc.declare_dram_parameter("parw8", [128, _W8_TOT], f8, isOutput=False)
    pars_d = nc.declare_dram_parameter("pars", [128, _S_TOT], f32, isOutput=False)
    out_d = nc.declare_dram_parameter("out", [BPC, 128, CT * HW], f32, isOutput=True)

    with tile.TileContext(nc) as tc:
        with (
            nc.allow_low_precision(reason="fp8 DoubleRow matmul pipeline by design"),
            tc.tile_pool(name="const", bufs=1) as const,
            tc.tile_pool(name="xp", bufs=4) as xp,
            tc.tile_pool(name="xnp", bufs=2) as xnp,
            tc.tile_pool(name="qkp", bufs=2) as qkp,
            tc.tile_pool(name="vp", bufs=2) as vp,
            tc.tile_pool(name="ptp", bufs=2) as ptp,
            tc.tile_pool(name="onp", bufs=2) as onp,
            tc.tile_pool(name="outp", bufs=2) as outp,
            tc.tile_pool(name="statp", bufs=2) as statp,
            tc.tile_pool(name="rbp", bufs=2) as rbp,
            tc.tile_pool(name="pS", bufs=2, space="PSUM") as pS,     # [128,512]
            tc.tile_pool(name="pF", bufs=1, space="PSUM") as pF,     # [128,2,512]
            tc.tile_pool(name="pob", bufs=3, space="PSUM") as pob,   # [128,512]
            tc.tile_pool(name="pgn", bufs=1, space="PSUM") as pgn,
        ):
            # ---- packed constants ----
            parw8_sb = const.tile([128, _W8_TOT], f8, name="parw8_sb")
            pars_sb = const.tile([128, _S_TOT], f32, name="pars_sb")
            m8_sb = parw8_sb[:, _W8_M : _W8_M + 512].rearrange("p (k f) -> p k f", f=256)
            wv_sb = parw8_sb[:, _W8_WV : _W8_WV + 512].rearrange("p (k f) -> p k f", f=256)
            wo_sb = parw8_sb[:, _W8_WO : _W8_WO + 512].rearrange("p (k f) -> p k f", f=256)
            ones_sb = parw8_sb[:, _W8_ONES : _W8_ONES + 256].rearrange("p (k f) -> p k f", f=128)
            wvec_sb = parw8_sb[:, _W8_WVEC : _W8_WVEC + 2].rearrange("p (k f) -> p k f", f=1)
            ind1_sb = pars_sb[:, _S_IND1 : _S_IND1 + 8]
            ind2_sb = pars_sb[0:4, _S_IND2 : _S_IND2 + 128]
            bo_sb = pars_sb[:, _S_BO : _S_BO + 2]
            gnwb_sb = pars_sb[:, _S_GNWB : _S_GNWB + 4].rearrange("p (k j) -> p k j", j=2)
            eps_sb = const.tile([128, 1], f32, name="eps_sb")
            nc.vector.memset(eps_sb, EPS)
            eb_sb = const.tile([128, 1], f32, name="eb_sb")
            nc.vector.memset(eb_sb, EXP_BIAS)

            # loop_n <= 8: python-unrolled; loop_n > 8: hardware For_i loop
            unroll, hw_loop = (loop_n, 1) if loop_n <= 8 else (1, loop_n)

            # Fixed-address state for the ROTATED batch 0: the body's bottom
            # (during batch 3's attention) prepares next iteration's batch-0
            # GN/front into these, so a For_i iteration never stalls on the
            # serial GroupNorm->z chain at its head. Iterations reload the
            # same DRAM x, so the refilled x0 content is identical.
            x0_sb = const.tile([128, CT * HW], f32, name="x0_sb")
            xn0_sb = const.tile([128, CT, HW], f8, name="xn0_sb")
            z0_sb = const.tile([128, CT, HW], f8, name="z0_sb")
            v0_sb = const.tile([128, TT, 256], f8, name="v0_sb")
            beta0_sb = const.tile([128, TT], f32, name="beta0_sb")

            gn = {}   # per-batch GN state: mv, ms, va, rs, ab, xn, beta
            qks, vs, ons, outs_sb = {}, {}, {}, {}
            xts = {0: x0_sb}
            seq = [0]

            def _u():
                seq[0] += 1
                return str(seq[0])

            if True:
                def emit_gn_stats(b):
                    """bn_stats/bn_aggr on DVE (first, ungated work)."""
                    u = f"{b}_{_u()}"
                    xv = xts[b].rearrange("p (k f) -> p k f", f=HW)
                    mv = statp.tile([128, CT, 2], f32, name=f"mv_{u}", tag="mv")
                    for kt in range(CT):
                        bnst = statp.tile([128, 2, 6], f32, name=f"bn_{u}_{kt}", tag="bnst")
                        xq = xv[:, kt, :].rearrange("p (a c) -> p a c", c=512)
                        for sg in range(2):
                            nc.vector.bn_stats(out=bnst[:, sg, :], in_=xq[:, sg, :])
                        nc.vector.bn_aggr(out=mv[:, kt, :], in_=bnst)
                    gn[b] = {"mv": mv}

                def emit_gn_mid(b):
                    """Indicator matmuls + group aggregation (Pool/PE/DVE)."""
                    u = f"{b}_{_u()}"
                    mv = gn[b]["mv"]
                    s12 = statp.tile([128, CT, 4], f32, name=f"s12_{u}", tag="s12")
                    pg = pgn.tile([4, 2 * 4], f32, name=f"pg_{u}", tag="gn")
                    nc.vector.tensor_copy(out=s12[:, :, 0:2], in_=mv)
                    nc.vector.tensor_copy(out=s12[:, :, 3:4], in_=mv[:, :, 0:1])
                    nc.vector.tensor_mul(s12[:, :, 2:3], mv[:, :, 0:1], mv[:, :, 0:1])
                    for kt in range(CT):
                        nc.tensor.matmul(
                            pg[:, 4 * kt : 4 * kt + 4],
                            ind1_sb[:, 4 * kt : 4 * kt + 4],
                            s12[:, kt, :],
                        )
                    gsum = statp.tile([4, 8], f32, name=f"gs_{u}", tag="gs")
                    nc.vector.tensor_copy(out=gsum, in_=pg)
                    ps2 = pgn.tile([128, CT, 4], f32, name=f"ps2_{u}", tag="gn")
                    for kt in range(CT):
                        nc.tensor.matmul(
                            ps2[:, kt, :], ind2_sb, gsum[:, 4 * kt : 4 * kt + 4]
                        )
                    # ms = [mean_g, E[var], E[mean^2], pad]; var = ms1+ms2-ms0^2
                    ms = statp.tile([128, CT, 4], f32, name=f"ms_{u}", tag="ms")
                    nc.vector.tensor_scalar_mul(out=ms, in0=ps2, scalar1=1.0 / GSZ)
                    va = statp.tile([128, CT, 1], f32, name=f"va_{u}", tag="va")
                    tmp = statp.tile([128, CT, 1], f32, name=f"tmp_{u}", tag="tmp")
                    nc.vector.tensor_add(va, ms[:, :, 1:2], ms[:, :, 2:3])
                    nc.vector.tensor_mul(tmp, ms[:, :, 0:1], ms[:, :, 0:1])
                    nc.vector.tensor_sub(va, va, tmp)
                    gn[b].update(ms=ms, va=va, tmp=tmp)

                def emit_gn_act(b):
                    """rsqrt via exp(-0.5*ln(var+eps)) on ACT, then the
                    per-channel scale/bias and the xn apply."""
                    u = f"{b}_{_u()}"
                    d = gn[b]
                    ms, va, tmp = d["ms"], d["va"], d["tmp"]
                    xv = xts[b].rearrange("p (k f) -> p k f", f=HW)
                    rs = statp.tile([128, CT, 1], f32, name=f"rs_{u}", tag="rs")
                    nc.scalar.activation(out=va, in_=va, func=AF.Ln, bias=eps_sb)
                    nc.scalar.activation(out=rs, in_=va, func=AF.Exp, scale=-0.5)
                    ab = statp.tile([128, CT, 2], f32, name=f"ab_{u}", tag="ab")
                    nc.vector.tensor_mul(ab[:, :, 0:1], gnwb_sb[:, :, 0:1], rs)
                    nc.vector.tensor_mul(tmp, ms[:, :, 0:1], ab[:, :, 0:1])
                    nc.vector.tensor_sub(ab[:, :, 1:2], gnwb_sb[:, :, 1:2], tmp)
                    xn_sb = xn0_sb if b == 0 else xnp.tile(
                        [128, CT, HW], f8, name=f"xn_{u}", tag="xn"
                    )
                    # both halves on DVE at 2x_2p: the Pool variant is 3x
                    # slower per op and sits on the xn->z critical chain
                    for kt in range(CT):
                        nc.vector.tensor_scalar(
                            out=xn_sb[:, kt, :],
                            in0=xv[:, kt, :],
                            scalar1=ab[:, kt, 0:1],
                            scalar2=ab[:, kt, 1:2],
                            op0=OP.mult,
                            op1=OP.add,
                        )
                    gn[b]["xn"] = xn_sb

                def emit_front_z(b):
                    """z = M xn (no bias needed) + per-t exp-bias beta."""
                    u = f"{b}_{_u()}"
                    xn_sb = gn[b]["xn"]
                    z_sb = z0_sb if b == 0 else qkp.tile(
                        [128, CT, HW], f8, name=f"z_{u}", tag="z"
                    )
                    qks[b] = z_sb
                    for m in range(CT):
                        pz = pF.tile([128, 2, 512], f32, name=f"pz_{u}_{m}", tag="f")
                        for n in range(NT):
                            nc.tensor.matmul(
                                pz[:, n, :],
                                m8_sb[:, :, 128 * m : 128 * m + 128],
                                xn_sb[:, :, 512 * n : 512 * n + 512],
                                start=True, stop=True, perf_mode=DR,
                            )
                        nc.vector.tensor_copy(
                            out=z_sb[:, m, :],
                            in_=pz.rearrange("p k f -> p (k f)"),
                        )
                    # beta[t] = (16 Wk^T bq).xn_t, folded with EXP_BIAS so the
                    # exp op's bias AP is simply beta_sb[:, t]
                    pb = pgn.tile([128, TT], f32, name=f"pb_{u}", tag="gn")
                    for t in range(TT):
                        nc.tensor.matmul(
                            pb[:, t : t + 1],
                            xn_sb[:, :, 128 * t : 128 * t + 128],
                            wvec_sb,
                            start=True, stop=True, perf_mode=DR,
                        )
                    beta = beta0_sb if b == 0 else statp.tile(
                        [128, TT], f32, name=f"beta_{u}", tag="beta"
                    )
                    nc.scalar.activation(
                        out=beta, in_=pb, func=AF.Identity,
                        scale=EXP_SCALE, bias=eb_sb,
                    )
                    gn[b]["beta"] = beta

                def emit_front_v(b):
                    """vT = xn^T WvT: 4 t-tiles per 2-bank psum, paired evac."""
                    u = f"{b}_{_u()}"
                    xn_sb = gn[b]["xn"]
                    v_sb = v0_sb if b == 0 else vp.tile(
                        [128, TT, 256], f8, name=f"v_{u}", tag="v"
                    )
                    vs[b] = v_sb
                    for g2 in range(2):
                        pv = pF.tile([128, 2, 512], f32, name=f"pv_{u}_{g2}", tag="f")
                        for gq in range(2):
                            for tq in range(2):
                                t = 4 * g2 + 2 * gq + tq
                                nc.tensor.matmul(
                                    pv[:, gq, 256 * tq : 256 * tq + 256],
                                    xn_sb[:, :, 128 * t : 128 * t + 128],
                                    wv_sb,
                                    start=True, stop=True, perf_mode=DR,
                                )
                        if g2 == 0:
                            nc.vector.tensor_copy(
                                out=v_sb[:, 4 * g2 : 4 * g2 + 4, :].rearrange(
                                    "p k f -> p (k f)"
                                ),
                                in_=pv.rearrange("p k f -> p (k f)"),
                            )
                        else:
                            nc.scalar.activation(
                                out=v_sb[:, 4 * g2 : 4 * g2 + 4, :].rearrange(
                                    "p k f -> p (k f)"
                                ),
                                in_=pv.rearrange("p k f -> p (k f)"),
                                func=AF.Copy,
                            )

                def emit_y(b, ns=(0, 1)):
                    """Deferred out-proj of batch b: y = Wout on / 256 + bo
                    (ACT/DVE alternating by m) then += x. Called with a single
                    n for the last batch so its n=0 half drains mid-attention."""
                    u = f"{b}_{_u()}"
                    last = b == BPC - 1
                    on_sb = ons[b]
                    o_sb = outs_sb.get(b)
                    if o_sb is None:
                        o_sb = outp.tile([128, CT * HW], f32, name=f"o_{u}", tag="out")
                        outs_sb[b] = o_sb
                    ov = o_sb.rearrange("p (k f) -> p k f", f=HW)
                    xv = xts[b].rearrange("p (k f) -> p k f", f=HW)
                    for m in range(CT):
                        for n in ns:
                            sl = slice(512 * n, 512 * n + 512)
                            pyt = pob.tile(
                                [128, 512], f32, name=f"py_{u}_{m}_{n}", tag="o"
                            )
                            nc.tensor.matmul(
                                pyt,
                                wo_sb[:, :, 128 * m : 128 * m + 128],
                                on_sb[:, :, sl],
                                start=True, stop=True, perf_mode=DR,
                            )
                            if m == 1 and last:
                                nc.vector.tensor_scalar(
                                    out=ov[:, m, sl], in0=pyt,
                                    scalar1=OUT_SCALE, scalar2=bo_sb[:, m : m + 1],
                                    op0=OP.mult, op1=OP.add,
                                )
                            else:
                                nc.scalar.activation(
                                    out=ov[:, m, sl], in_=pyt, func=AF.Identity,
                                    scale=OUT_SCALE, bias=bo_sb[:, m : m + 1],
                                )
                            if last:
                                eng = nc.vector if m == 0 else nc.gpsimd
                                eng.tensor_add(
                                    ov[:, m, sl], ov[:, m, sl], xv[:, m, sl]
                                )
                                lo = HW * m + 512 * n
                                nc.sync.dma_start(
                                    out=out_d[b][:, lo : lo + 512],
                                    in_=o_sb[:, lo : lo + 512],
                                )
                        if not last:
                            nc.gpsimd.tensor_add(
                                ov[:, m, :], ov[:, m, :], xv[:, m, :]
                            )
                    if not last:
                        nc.sync.dma_start(out=out_d[b][:, :], in_=o_sb)

                def emit_prologue():
                    # cold-start batch-0 state; quarters let bn_stats start
                    # as each lands, constants interleave behind
                    for q in range(4):
                        nc.sync.dma_start(
                            out=x0_sb[:, 512 * q : 512 * q + 512],
                            in_=x_d[0][:, 512 * q : 512 * q + 512],
                        )
                        if q == 1:
                            nc.sync.dma_start(out=pars_sb, in_=pars_d[:, :])
                    nc.sync.dma_start(out=parw8_sb, in_=parw8_d[:, :])
                    emit_gn_stats(0)
                    emit_gn_mid(0)
                    emit_gn_act(0)
                    emit_front_z(0)
                    emit_front_v(0)

                def emit_body():
                  outs_sb.clear()
                  for b in range(1, BPC):
                    x_sb = xp.tile([128, CT * HW], f32, name=f"x_{b}_{_u()}", tag="x")
                    xts[b] = x_sb
                    nc.sync.dma_start(out=x_sb, in_=x_d[b])
                  for b in range(BPC):
                    u = f"{b}_{_u()}"
                    z_sb = qks[b]
                    xn_b = gn[b]["xn"]
                    beta_b = gn[b]["beta"]
                    v_sb = vs[b]
                    nxt = (b + 1) % BPC
                    if b + 1 == BPC:
                        # refill x0 (same bytes) and rotate next iteration's
                        # batch-0 prep under this batch's attention
                        nc.sync.dma_start(out=x0_sb, in_=x_d[0])
                    emit_gn_stats(nxt)
                    on_sb = onp.tile([128, CT, HW], f8, name=f"on_{u}", tag="on")
                    ons[b] = on_sb
                    pts = ptp.tile([128, TT, HW], f8, name=f"pT_{u}", tag="pT")
                    po = {}

                    def alloc_o(n):
                        po[0, n] = pob.tile([128, 512], f32, name=f"po0_{u}_{n}", tag="o")
                        po[1, n] = pob.tile([128, 512], f32, name=f"po1_{u}_{n}", tag="o")
                        po[2, n] = pob.tile([128, 512], f32, name=f"pd_{u}_{n}", tag="o")

                    def emit_o(gp, n):
                        st, sp = (gp == 0), (gp == TT // 2 - 1)
                        rhs = pts[:, 2 * gp : 2 * gp + 2, 512 * n : 512 * n + 512]
                        nc.tensor.matmul(po[0, n], v_sb[:, 2 * gp : 2 * gp + 2, 0:128],
                                         rhs, start=st, stop=sp, perf_mode=DR)
                        nc.tensor.matmul(po[1, n], v_sb[:, 2 * gp : 2 * gp + 2, 128:256],
                                         rhs, start=st, stop=sp, perf_mode=DR)
                        nc.tensor.matmul(po[2, n], ones_sb,
                                         rhs, start=st, stop=sp, perf_mode=DR)

                    def s_exp(t, n):
                        psT = pS.tile([128, 512], f32, name=f"pS_{u}_{t}_{n}", tag="s")
                        nc.tensor.matmul(
                            psT,
                            xn_b[:, :, 128 * t : 128 * t + 128],
                            z_sb[:, :, 512 * n : 512 * n + 512],
                            start=True, stop=True, perf_mode=DR,
                        )
                        nc.scalar.activation(
                            out=pts[:, t, 512 * n : 512 * n + 512], in_=psT,
                            func=AF.Exp, scale=EXP_SCALE, bias=beta_b[:, t : t + 1],
                        )

                    def normalize(n):
                        rb = rbp.tile([128, 512], f32, name=f"rb_{u}_{n}", tag="rb")
                        nc.vector.reciprocal_approx_fast(out=rb, in_=po[2, n])
                        nc.vector.tensor_mul(
                            on_sb[:, 0, 512 * n : 512 * n + 512], po[0, n], rb
                        )
                        nc.vector.tensor_mul(
                            on_sb[:, 1, 512 * n : 512 * n + 512], po[1, n], rb
                        )

                    alloc_o(0)
                    for t in range(TT):
                        s_exp(t, 0)
                        if t == 2:
                            emit_gn_mid(nxt)
                        if t == 4:
                            emit_gn_act(nxt)
                        if t >= 3 and t % 2 == 1:
                            emit_o((t - 3) // 2, 0)
                    emit_o(TT // 2 - 1, 0)
                    # z(nxt) evacs go ahead of normalize in the DVE queue so
                    # they run while this batch's exps stream on ACT
                    emit_front_z(nxt)
                    normalize(0)
                    if b == BPC - 1:
                        emit_y(b, ns=(0,))
                    alloc_o(1)
                    for t in range(TT):
                        s_exp(t, 1)
                        if t >= 3 and t % 2 == 1:
                            emit_o((t - 3) // 2, 1)
                    emit_o(TT // 2 - 1, 1)
                    normalize(1)
                    emit_front_v(nxt)
                    if b > 0:
                        emit_y(b - 1)
                  emit_y(BPC - 1, ns=(1,))

                emit_prologue()
                if hw_loop == 1:
                    for _ in range(unroll):
                        emit_body()
                else:
                    with tc.For_i(0, hw_loop, 1):
                        emit_body()
    nc.compile()
    return nc


def _host_inputs(x, gn_weight, gn_bias, w_qkv, b_qkv, w_out, b_out):
    """Fold/reshape parameters into the packed layout; shard x."""
    import ml_dtypes

    f = np.float32
    f8 = ml_dtypes.float8_e4m3fn
    x = np.ascontiguousarray(x, dtype=f).reshape(B, C, HW)
    wq = w_qkv[0:256].astype(f)
    wk = w_qkv[256:512].astype(f)
    wv = w_qkv[512:768].astype(f)
    m8T = (f(SC) * (wk.T @ wq)).T                           # (256, 256): z = M xn
    wvT = wv.T * f(SC)                                      # (256, 256)
    woT = w_out.astype(f).T * f(SC)                         # (256, 256)
    bq = b_qkv[0:256].astype(f)
    bv = b_qkv[512:768].astype(f)
    wvec = f(SC) * (wk.T @ bq)                              # (256,)
    bo = b_out.astype(f) + w_out.astype(f) @ bv             # (256,)

    parw8 = np.zeros((128, _W8_TOT), dtype=f8)
    pars = np.zeros((128, _S_TOT), dtype=f)
    for kt in range(CT):
        sl = slice(128 * kt, 128 * kt + 128)
        parw8[:, _W8_M + 256 * kt : _W8_M + 256 * kt + 256] = m8T[sl].astype(f8)
        parw8[:, _W8_WV + 256 * kt : _W8_WV + 256 * kt + 256] = wvT[sl].astype(f8)
        parw8[:, _W8_WO + 256 * kt : _W8_WO + 256 * kt + 256] = woT[sl].astype(f8)
        parw8[:, _W8_ONES + 128 * kt : _W8_ONES + 128 * kt + 128] = f8(1.0)
        parw8[:, _W8_WVEC + kt] = wvec[sl].astype(f8)
        pars[:, _S_BO + kt] = bo[sl]
        pars[:, _S_GNWB + 2 * kt] = gn_weight.astype(f)[sl]
        pars[:, _S_GNWB + 2 * kt + 1] = gn_bias.astype(f)[sl]
    for gl in range(4):
        pars[32 * gl : 32 * gl + 32, _S_IND1 + gl] = 1.0
        pars[32 * gl : 32 * gl + 32, _S_IND1 + 4 + gl] = 1.0
    for cc in range(128):
        pars[cc // 32, _S_IND2 + cc] = 1.0

    in_maps = []
    for i in range(NCORES):
        xs = x[BPC * i : BPC * (i + 1)].reshape(BPC, CT, 128, HW)
        xs = np.ascontiguousarray(xs.transpose(0, 2, 1, 3).reshape(BPC, 128, CT * HW))
        in_maps.append({"x": xs, "parw8": parw8, "pars": pars})
    return in_maps


def kernel(x, gn_weight, gn_bias, w_qkv, b_qkv, w_out, b_out):
    from concourse.bass_utils import run_bass_kernel_spmd

    in_maps = _host_inputs(x, gn_weight, gn_bias, w_qkv, b_qkv, w_out, b_out)
    nc = _build(_LOOP_N)
    res = run_bass_kernel_spmd(nc, in_maps, list(range(NCORES)))
    outs = []
    for i in range(NCORES):
        o = res.results[i]["out"].reshape(BPC, 128, CT, HW)
        outs.append(o.transpose(0, 2, 1, 3).reshape(BPC, C, HW))
    return np.concatenate(outs).reshape(B, C, H, W).astype(np.float32)
